# revision 3
# baseline (speedup 1.0000x reference)
# Trainium2 Bass kernel for nn_CustomGate: y = (I_L (x) M (x) I_R) @ x
# with D=2, N=13, INDEX=5 -> L=32, R=128, DIM=8192, BATCH=2048, complex64.
#
# Math: viewing x as [L, D, R, B], the gate mixes only the D axis:
#   y[l, a, r, b] = sum_b' M[a, b'] x[l, b', r, b]
# Splitting complex into real/imag gives, per (l, r, b), a fixed real 4x4
# mix A = [[Mr, -Mi], [Mi, Mr]] over components (x0r, x1r, x0i, x1i).
#
# Sharding: L axis across 8 cores -> core i owns rows [1024*i, 1024*(i+1))
# of x_real/x_imag (contiguous slabs, no cross-core communication).
#
# Per core, for each 256-row l-block ([d(2), q(32), rl(4)] rows x b(2048)):
# DMA into one SBUF tile [128, 8192] with partition p = comp*32 + q
# (comp in {x0r, x1r, x0i, x1i}) and free = rl*2048 + b. A single fp32
# TensorE matmul per 512-col chunk against the stationary
# W = A^T (x) I_32 (host-precomputed, [128, 128]) produces all 4 output
# components in one pass. PSUM is evicted to SBUF (DVE/ACT alternating)
# and DMA'd back out with the mirrored access pattern.

import numpy as np

N_CORES = 8
DIM = 8192
BATCH = 2048
ROWS_PER_CORE = DIM // N_CORES  # 1024
L_BLOCKS_PER_CORE = ROWS_PER_CORE // 256  # 4
FREE = 4 * BATCH  # 8192 free elements per l-block tile
JCH = 512  # matmul free-dim chunk (one PSUM bank of fp32)
NJ = FREE // JCH  # 16

_PROGRAM = None


def _build_program():
    import concourse.bacc as bacc
    import concourse.tile as tile
    import concourse.mybir as mybir

    F32 = mybir.dt.float32

    # Bacc (not raw Bass): its compile() runs move_matmul_waits_to_ldweights
    # + generate_event_semaphores, which legalize multi-wait instructions for
    # TRN2 (at most 1 sync wait per instruction).
    nc = bacc.Bacc("TRN2", target_bir_lowering=False)
    w = nc.declare_dram_parameter("w", [128, 128], F32, isOutput=False)
    xr = nc.declare_dram_parameter("xr", [ROWS_PER_CORE, BATCH], F32, isOutput=False)
    xi = nc.declare_dram_parameter("xi", [ROWS_PER_CORE, BATCH], F32, isOutput=False)
    yr = nc.declare_dram_parameter("yr", [ROWS_PER_CORE, BATCH], F32, isOutput=True)
    yi = nc.declare_dram_parameter("yi", [ROWS_PER_CORE, BATCH], F32, isOutput=True)

    def shuffled(t, l):
        # [256, 2048] slab -> [(d q), (rl b)] = [64, 8192]
        return t[l * 256 : (l + 1) * 256, :].rearrange(
            "(d q rl) b -> (d q) (rl b)", d=2, q=32, rl=4
        )

    with tile.TileContext(nc) as tc:
        with (
            tc.tile_pool(name="wpool", bufs=1) as wpool,
            tc.tile_pool(name="inpool", bufs=2) as inpool,
            tc.tile_pool(name="outpool", bufs=2) as outpool,
            tc.tile_pool(name="psum", bufs=8, space="PSUM") as psumpool,
        ):
            wt = wpool.tile([128, 128], F32)
            nc.sync.dma_start(out=wt[:], in_=w[:])
            for l in range(L_BLOCKS_PER_CORE):
                xt = inpool.tile([128, FREE], F32, tag="xt")
                nc.sync.dma_start(out=xt[0:64, :], in_=shuffled(xr, l))
                nc.sync.dma_start(out=xt[64:128, :], in_=shuffled(xi, l))
                yt = outpool.tile([128, FREE], F32, tag="yt")
                for j in range(NJ):
                    ps = psumpool.tile([128, JCH], F32)
                    nc.tensor.matmul(
                        ps[:],
                        lhsT=wt[:],
                        rhs=xt[:, j * JCH : (j + 1) * JCH],
                        start=True,
                        stop=True,
                    )
                    if j % 2 == 0:
                        nc.vector.tensor_copy(yt[:, j * JCH : (j + 1) * JCH], ps[:])
                    else:
                        nc.scalar.copy(yt[:, j * JCH : (j + 1) * JCH], ps[:])
                nc.sync.dma_start(out=shuffled(yr, l), in_=yt[0:64, :])
                nc.sync.dma_start(out=shuffled(yi, l), in_=yt[64:128, :])
    nc.compile()
    return nc


def _get_program():
    global _PROGRAM
    if _PROGRAM is None:
        _PROGRAM = _build_program()
    return _PROGRAM


def _make_w(M_real, M_imag):
    Mr = np.asarray(M_real, dtype=np.float32)
    Mi = np.asarray(M_imag, dtype=np.float32)
    # components in = (x0r, x1r, x0i, x1i), out = (y0r, y1r, y0i, y1i)
    A = np.block([[Mr, -Mi], [Mi, Mr]]).astype(np.float32)  # [4, 4]
    # matmul computes out[i, j] = sum_k W[k, i] rhs[k, j]; k/i = (comp, q)
    W = np.kron(A.T, np.eye(32, dtype=np.float32)).astype(np.float32)
    return np.ascontiguousarray(W)


def _in_maps(W, x_real, x_imag):
    maps = []
    for i in range(N_CORES):
        sl = slice(i * ROWS_PER_CORE, (i + 1) * ROWS_PER_CORE)
        maps.append(
            {
                "w": W,
                "xr": np.ascontiguousarray(x_real[sl]),
                "xi": np.ascontiguousarray(x_imag[sl]),
            }
        )
    return maps


def kernel(M_real, M_imag, x_real, x_imag):
    from concourse import bass_utils

    x_real = np.asarray(x_real, dtype=np.float32)
    x_imag = np.asarray(x_imag, dtype=np.float32)
    W = _make_w(M_real, M_imag)

    nc = _get_program()
    res = bass_utils.run_bass_kernel_spmd(
        nc, _in_maps(W, x_real, x_imag), list(range(N_CORES))
    )

    y = np.empty((DIM, BATCH), dtype=np.complex64)
    for i in range(N_CORES):
        sl = slice(i * ROWS_PER_CORE, (i + 1) * ROWS_PER_CORE)
        y.real[sl] = res.results[i]["yr"]
        y.imag[sl] = res.results[i]["yi"]
    return y


# revision 4
# speedup vs baseline: 1.6332x; 1.6332x over previous
# Trainium2 Bass kernel for nn_CustomGate: y = (I_L (x) M (x) I_R) @ x
# with D=2, N=13, INDEX=5 -> L=32, R=128, DIM=8192, BATCH=2048, complex64.
#
# Math: viewing x as [L, D, R, B], the gate mixes only the D axis:
#   y[l, a, r, b] = sum_b' M[a, b'] x[l, b', r, b]
# Splitting complex into real/imag gives, per (l, r, b), a fixed real 4x4
# mix A = [[Mr, -Mi], [Mi, Mr]] over components (x0r, x1r, x0i, x1i).
#
# Sharding: L axis across 8 cores -> core i owns rows [1024*i, 1024*(i+1))
# of x_real/x_imag (contiguous slabs, no cross-core communication).
#
# The host pre-interleaves each core's slab into xcat [128, 4*8192] fp32:
# partition p = comp*32 + q (comp in {x0r, x1r, x0i, x1i}, q = r_hi) and
# free = l*8192 + rl*2048 + b (r = q*4 + rl). Device DMAs are then fully
# contiguous [128, 32KB] slabs. One fp32 TensorE matmul per 512-col chunk
# against the stationary W = A^T (x) I_32 (host-precomputed, [128, 128])
# produces all 4 output components in one pass. PSUM is evicted to SBUF
# (DVE/ACT alternating) and DMA'd out contiguously (separate HWDGE ring
# from the input DMAs), then the host de-interleaves.

import numpy as np

N_CORES = 8
DIM = 8192
BATCH = 2048
ROWS_PER_CORE = DIM // N_CORES  # 1024
NL = ROWS_PER_CORE // 256  # 4 l-blocks per core
FREE = 4 * BATCH  # 8192 free elements per l-block tile
JCH = 512  # matmul free-dim chunk (one PSUM bank of fp32)
NJ = FREE // JCH  # 16

_PROGRAM = None


def _build_program():
    import concourse.bacc as bacc
    import concourse.tile as tile
    import concourse.mybir as mybir

    F32 = mybir.dt.float32

    # Bacc (not raw Bass): its compile() runs move_matmul_waits_to_ldweights
    # + generate_event_semaphores, which legalize multi-wait instructions for
    # TRN2 (at most 1 sync wait per instruction).
    nc = bacc.Bacc("TRN2", target_bir_lowering=False)
    w = nc.declare_dram_parameter("w", [128, 128], F32, isOutput=False)
    xin = nc.declare_dram_parameter("xin", [128, NL * FREE], F32, isOutput=False)
    yout = nc.declare_dram_parameter("yout", [128, NL * FREE], F32, isOutput=True)

    with tile.TileContext(nc) as tc:
        with (
            tc.tile_pool(name="wpool", bufs=1) as wpool,
            tc.tile_pool(name="inpool", bufs=3) as inpool,
            tc.tile_pool(name="outpool", bufs=2) as outpool,
            tc.tile_pool(name="psum", bufs=8, space="PSUM") as psumpool,
        ):
            wt = wpool.tile([128, 128], F32)
            nc.sync.dma_start(out=wt[:], in_=w[:])
            for l in range(NL):
                xt = inpool.tile([128, FREE], F32, tag="xt")
                nc.sync.dma_start(out=xt[:], in_=xin[:, l * FREE : (l + 1) * FREE])
                yt = outpool.tile([128, FREE], F32, tag="yt")
                for j in range(NJ):
                    ps = psumpool.tile([128, JCH], F32)
                    nc.tensor.matmul(
                        ps[:],
                        lhsT=wt[:],
                        rhs=xt[:, j * JCH : (j + 1) * JCH],
                        start=True,
                        stop=True,
                    )
                    if j % 2 == 0:
                        nc.vector.tensor_copy(yt[:, j * JCH : (j + 1) * JCH], ps[:])
                    else:
                        nc.scalar.copy(yt[:, j * JCH : (j + 1) * JCH], ps[:])
                # output on the ACT HWDGE ring so input/output DMAs round-robin
                # on the SDMA engines instead of queuing FIFO behind each other
                nc.scalar.dma_start(
                    out=yout[:, l * FREE : (l + 1) * FREE], in_=yt[:]
                )
    nc.compile()
    return nc


def _get_program():
    global _PROGRAM
    if _PROGRAM is None:
        _PROGRAM = _build_program()
    return _PROGRAM


def _make_w(M_real, M_imag):
    Mr = np.asarray(M_real, dtype=np.float32)
    Mi = np.asarray(M_imag, dtype=np.float32)
    # components in = (x0r, x1r, x0i, x1i), out = (y0r, y1r, y0i, y1i)
    A = np.block([[Mr, -Mi], [Mi, Mr]]).astype(np.float32)  # [4, 4]
    # matmul computes out[i, j] = sum_k W[k, i] rhs[k, j]; k/i = (comp, q)
    W = np.kron(A.T, np.eye(32, dtype=np.float32)).astype(np.float32)
    return np.ascontiguousarray(W)


def _interleave(slab):
    # [1024, 2048] -> [64, 4*8192]: [l, d, q, rl, b] -> [(d q), (l rl b)]
    xs = slab.reshape(NL, 2, 32, 4, BATCH)
    return xs.transpose(1, 2, 0, 3, 4).reshape(64, NL * FREE)


def _deinterleave(half):
    # [64, 4*8192] -> [1024, 2048]
    ys = half.reshape(2, 32, NL, 4, BATCH)
    return ys.transpose(2, 0, 1, 3, 4).reshape(ROWS_PER_CORE, BATCH)


def _in_maps(W, x_real, x_imag):
    maps = []
    for i in range(N_CORES):
        sl = slice(i * ROWS_PER_CORE, (i + 1) * ROWS_PER_CORE)
        xcat = np.empty((128, NL * FREE), dtype=np.float32)
        xcat[0:64] = _interleave(x_real[sl])
        xcat[64:128] = _interleave(x_imag[sl])
        maps.append({"w": W, "xin": xcat})
    return maps


def _gather(results):
    y = np.empty((DIM, BATCH), dtype=np.complex64)
    for i in range(N_CORES):
        sl = slice(i * ROWS_PER_CORE, (i + 1) * ROWS_PER_CORE)
        ycat = results[i]["yout"]
        y.real[sl] = _deinterleave(ycat[0:64])
        y.imag[sl] = _deinterleave(ycat[64:128])
    return y


def kernel(M_real, M_imag, x_real, x_imag):
    from concourse import bass_utils

    x_real = np.asarray(x_real, dtype=np.float32)
    x_imag = np.asarray(x_imag, dtype=np.float32)
    W = _make_w(M_real, M_imag)

    nc = _get_program()
    res = bass_utils.run_bass_kernel_spmd(
        nc, _in_maps(W, x_real, x_imag), list(range(N_CORES))
    )
    return _gather(res.results)
